# revision 4
# baseline (speedup 1.0000x reference)
"""Trainium2 Bass kernel for causal bilinear self-attention (diagonal variant).

Computes, per (b, head):
    scores[t, s] = h[b, t] @ A[head] @ h[b, s]        (causal: s <= t)
    attn = softmax(scores, axis=-1)
    out[b, head, t, :] = attn[t, t] * h[b, t, :]
returned reshaped row-major to (B, T, H*d)  (faithful torch .view semantics).

Only the softmax DIAGONAL is needed:  attn[t,t] = exp(s_tt) / sum_{s<=t} exp(s_ts).
Instead of the classic max-stabilized softmax, the kernel shifts each row by its
own DIAGONAL score:  attn[t,t] = 1 / sum_{s<=t} exp(s_ts - s_tt).
  - numerator is exactly exp(0) = 1 (the shift is the bitwise-identical stored
    diagonal score, so the diag term of the sum is also exactly 1);
  - the sum is therefore >= 1: reciprocal never divides by ~0, no NaN path;
  - rows where some s_ts exceeds s_tt by > ~88 overflow exp to +inf, giving
    attn[t,t] = 1/inf = 0, which matches the fp32 reference (there
    exp(s_tt - max) underflows to exactly 0).
This removes ALL per-chunk reduce_max work from the DVE, which would otherwise
be the bottleneck.

Precision: f32r (TF32-like, 11-bit mantissa) single-pass matmuls at 1 cyc/row,
~1.4e-3 max rel error vs the 2e-2 gate. f32 inputs are DMA'd / copied as raw
bits into f32r-typed tiles (PE rounds on read), so no rounding passes needed.

Causal masking of the diagonal 128x128 block is done INSIDE PSUM accumulation
by one extra bf16 matmul (identity @ cmask adds -1e30 to the upper triangle),
so ACT can exp straight out of PSUM and the DVE never touches scores except to
extract the diagonal (one 128-wide copy + one tensor_tensor_reduce per row
tile).

Sharding: 16 (b, head) pairs across 8 cores -> core c handles b = c // 4,
heads 2*(c%4) and 2*(c%4)+1.  Each core receives h[b] (4 MB) and its two
A matrices; outputs its two (T, d) slices.

Hardware notes (inherited from previous sessions on this toolchain):
  - tensor_tensor_reduce with a PSUM input crashes the device; PSUM is written
    only by PE/ACT here and DVE only copy-reads it.
  - mask/identity constants are DMA'd from host inputs.
"""

import sys

try:
    import concourse.bass  # noqa: F401
except ImportError:  # pragma: no cover
    sys.path.insert(0, "/opt/trn_rl_repo")

import numpy as np

import concourse.bass as bass  # noqa: F401
import concourse.tile as tile
from concourse import bacc, bass_utils, mybir

B, T, D, H = 2, 2048, 512, 8
NCORES = 8
P = 128
NT = T // P      # 16 row tiles
ND = D // P      # 4 contraction chunks
SCH = 512        # s-chunk width (one PSUM bank of fp32)
NS = T // SCH    # 4 column chunks
NEG = -1.0e30

f32 = mybir.dt.float32
f32r = mybir.dt.float32r
bf16 = mybir.dt.bfloat16
EXP = mybir.ActivationFunctionType.Exp
MULT = mybir.AluOpType.mult
ADD = mybir.AluOpType.add
AXX = mybir.AxisListType.X


def build_nc():
    nc = bacc.Bacc("TRN2", target_bir_lowering=False, debug=False)
    hb = nc.dram_tensor("hb", [T, D], f32, kind="ExternalInput")
    A2 = nc.dram_tensor("A2", [2, D, D], f32, kind="ExternalInput")
    cmaskd = nc.dram_tensor("cmaskd", [P, P], f32, kind="ExternalInput")
    identd = nc.dram_tensor("identd", [P, P], f32, kind="ExternalInput")
    out2 = nc.dram_tensor("out2", [2, T, D], f32, kind="ExternalOutput")
    hb_t = hb[:].rearrange("(n p) d -> p n d", p=P)      # [128, 16, 512]
    A2r = A2[:].rearrange("h (c p) e -> p h c e", p=P)   # [128, 2, 4, 512]

    with tile.TileContext(nc) as tc:
        with (
            tc.tile_pool(name="const", bufs=1) as constp,
            tc.tile_pool(name="big", bufs=1) as big,
            tc.tile_pool(name="psum", bufs=8, space="PSUM") as psum,
            tc.tile_pool(name="scr", bufs=3) as scr,
            tc.tile_pool(name="stats", bufs=6) as stats,
            tc.tile_pool(name="outp", bufs=3) as outp,
        ):
            ident = constp.tile([P, P], f32)
            nc.gpsimd.dma_start(out=ident, in_=identd[:])
            cmaskf = constp.tile([P, P], f32)
            nc.gpsimd.dma_start(out=cmaskf, in_=cmaskd[:])
            identb = constp.tile([P, P], bf16)
            nc.vector.tensor_copy(identb, ident)
            cmaskb = constp.tile([P, P], bf16)
            nc.vector.tensor_copy(cmaskb, cmaskf)
            ident_r = ident.bitcast(f32r)

            # The sim serializes all DMA transfers on one device, so emission
            # order ~= arrival order: h rows 0-3 first (transposes start),
            # then A (stage1), then the rest of h.
            A_r = big.tile([P, 2, ND, D], f32r)
            h_all = big.tile([P, NT, D], f32)
            dmaq = [nc.sync, nc.scalar]
            for i in range(4):
                dmaq[i % 2].dma_start(out=h_all[:, i, :], in_=hb_t[:, i, :])
            # A as f32r bits (PE rounds on read): A_r[p, hd, c, e] = A[hd, c*128+p, e]
            for hd in range(2):
                for half in range(2):
                    dmaq[half].dma_start(
                        out=A_r[:, hd, 2 * half : 2 * half + 2].bitcast(f32),
                        in_=A2r[:, hd, 2 * half : 2 * half + 2],
                    )
            for i in range(4, NT):
                dmaq[i % 2].dma_start(out=h_all[:, i, :], in_=hb_t[:, i, :])

            # h^T: hTr[p, c, t] = h[t, c*128 + p]   (f32r bits)
            hTr = big.tile([P, ND, T], f32r)
            # gT = A^T @ hT per head: gTh[p, ec, t] = g[t, ec*128 + p]
            gTh = [big.tile([P, ND, T], f32r, name=f"gTh{hd}") for hd in range(2)]

            def transpose_rowtile(i):
                pt = psum.tile([P, ND, P], f32r, tag="ps")
                for c in range(ND):
                    nc.tensor.transpose(
                        pt[:, c, :],
                        h_all[:, i, c * P : (c + 1) * P].bitcast(f32r),
                        ident_r,
                    )
                # one 512-wide copy distributes the 4 chunks into hTr
                nc.vector.tensor_copy(
                    hTr[:, :, i * P : (i + 1) * P].bitcast(f32), pt.bitcast(f32)
                )

            def stage1(hd, tsl):
                ts_ = slice(tsl * SCH, (tsl + 1) * SCH)
                for ec in range(ND):
                    pg = psum.tile([P, SCH], f32, tag="ps")
                    for k in range(ND):
                        nc.tensor.matmul(
                            pg,
                            A_r[:, hd, k, ec * P : (ec + 1) * P],
                            hTr[:, k, ts_],
                            start=(k == 0),
                            stop=(k == ND - 1),
                        )
                    nc.vector.tensor_copy(gTh[hd][:, ec, ts_].bitcast(f32), pg)

            def stage2_rowtile(hd, i):
                nch = i // 4 + 1
                its = slice(i * P, (i + 1) * P)
                dcol = (i % 4) * P
                wlast = dcol + P          # causal width within last chunk
                w_mm = max(wlast, 2 * P)  # f32r needs moving dim >= 256
                lp = stats.tile([P, 4], f32, tag="lp")
                negd = stats.tile([P, 1], f32, tag="negd")
                chunks = []
                # diag chunk FIRST so negd is ready while PE does the rest
                for idx, j in enumerate([nch - 1] + list(range(nch - 1))):
                    last = j == nch - 1
                    w = w_mm if last else SCH
                    wc = wlast if last else SCH
                    ps = psum.tile([P, SCH], f32, tag="ps")
                    for k in range(ND):
                        nc.tensor.matmul(
                            ps[:, :w],
                            gTh[hd][:, k, its],
                            hTr[:, k, j * SCH : j * SCH + w],
                            start=(k == 0),
                            stop=(k == ND - 1 and not last),
                        )
                    if last:
                        # causal mask added inside PSUM: += I @ cmask
                        nc.tensor.matmul(
                            ps[:, dcol : dcol + P],
                            identb,
                            cmaskb,
                            start=False,
                            stop=True,
                        )
                        # extract -s_tt (diag of the masked block)
                        scd = scr.tile([P, P], f32, tag="scd")
                        nc.vector.tensor_copy(scd, ps[:, dcol : dcol + P])
                        tto = scr.tile([P, P], f32, tag="tto")
                        nc.vector.tensor_tensor_reduce(
                            out=tto,
                            in0=scd,
                            in1=ident,
                            scale=-1.0,
                            scalar=0.0,
                            op0=MULT,
                            op1=ADD,
                            accum_out=negd,
                        )
                    chunks.append((ps, wc, idx))
                for ps, wc, idx in chunks:
                    nc.scalar.activation(
                        out=ps[:, :wc],
                        in_=ps[:, :wc],
                        func=EXP,
                        bias=negd,
                        scale=1.0,
                        accum_out=lp[:, idx : idx + 1],
                    )
                lsum = stats.tile([P, 1], f32, tag="lsum")
                nc.vector.reduce_sum(out=lsum, in_=lp[:, :nch], axis=AXX)
                datt = stats.tile([P, 1], f32, tag="datt")
                nc.vector.reciprocal(datt, lsum)
                ot = outp.tile([P, D], f32, tag="ot")
                nc.vector.tensor_scalar_mul(ot, h_all[:, i, :], datt)
                nc.sync.dma_start(out=out2[hd, its, :], in_=ot)

            # interleaved schedule: group g+1's transposes+stage1 are emitted
            # BEFORE group g's stage2 rows so the DVE hTr/g copies stay ahead
            # of the PE matmul stream
            def group_ts1(tsl):
                for i in range(4 * tsl, 4 * tsl + 4):
                    transpose_rowtile(i)
                for hd in range(2):
                    stage1(hd, tsl)

            group_ts1(0)
            for tsl in range(NS):
                if tsl + 1 < NS:
                    group_ts1(tsl + 1)
                for hd in range(2):
                    for i in range(4 * tsl, 4 * tsl + 4):
                        stage2_rowtile(hd, i)

    nc.compile()
    return nc


_NC_CACHE = {}


def _get_nc():
    if "nc" not in _NC_CACHE:
        _NC_CACHE["nc"] = build_nc()
    return _NC_CACHE["nc"]


def _consts():
    cmask = np.triu(np.full((P, P), NEG, np.float32), 1)
    ident = np.eye(P, dtype=np.float32)
    return cmask, ident


def make_in_maps(h, A):
    h = np.ascontiguousarray(h, dtype=np.float32)
    A = np.ascontiguousarray(A, dtype=np.float32)
    cmask, ident = _consts()
    in_maps = []
    for c in range(NCORES):
        b = c // 4
        h0 = 2 * (c % 4)
        in_maps.append({"hb": h[b], "A2": np.ascontiguousarray(A[h0 : h0 + 2]),
                        "cmaskd": cmask, "identd": ident})
    return in_maps


def assemble(results):
    full = np.empty((B, H, T, D), dtype=np.float32)
    for c in range(NCORES):
        b = c // 4
        h0 = 2 * (c % 4)
        o = results[c]["out2"]
        full[b, h0] = o[0]
        full[b, h0 + 1] = o[1]
    return full.reshape(B, T, H * D)


def kernel(h, A):
    nc = _get_nc()
    res = bass_utils.run_bass_kernel_spmd(
        nc, make_in_maps(h, A), core_ids=list(range(NCORES))
    )
    return assemble(res.results)


# revision 5
# speedup vs baseline: 1.2419x; 1.2419x over previous
"""Trainium2 Bass kernel for causal bilinear self-attention (diagonal variant).

Computes, per (b, head):
    scores[t, s] = h[b, t] @ A[head] @ h[b, s]        (causal: s <= t)
    attn = softmax(scores, axis=-1)
    out[b, head, t, :] = attn[t, t] * h[b, t, :]
returned reshaped row-major to (B, T, H*d)  (faithful torch .view semantics).

Only the softmax DIAGONAL is needed:  attn[t,t] = exp(s_tt) / sum_{s<=t} exp(s_ts).
Instead of the classic max-stabilized softmax, the kernel shifts each row by its
own DIAGONAL score:  attn[t,t] = 1 / sum_{s<=t} exp(s_ts - s_tt).
  - numerator is exactly exp(0) = 1 (the shift is the bitwise-identical stored
    diagonal score, so the diag term of the sum is also exactly 1);
  - the sum is therefore >= 1: reciprocal never divides by ~0, no NaN path;
  - rows where some s_ts exceeds s_tt by > ~88 overflow exp to +inf, giving
    attn[t,t] = 1/inf = 0, which matches the fp32 reference (there
    exp(s_tt - max) underflows to exactly 0).
This removes ALL per-chunk reduce_max work from the DVE, which would otherwise
be the bottleneck.

Precision: f32r (TF32-like, 11-bit mantissa) single-pass matmuls at 1 cyc/row,
~1.4e-3 max rel error vs the 2e-2 gate. f32 inputs are DMA'd / copied as raw
bits into f32r-typed tiles (PE rounds on read), so no rounding passes needed.

Causal masking of the diagonal 128x128 block is done INSIDE PSUM accumulation
by one extra bf16 matmul (identity @ cmask adds -1e30 to the upper triangle),
so ACT can exp straight out of PSUM and the DVE never touches scores except to
extract the diagonal (one 128-wide copy + one tensor_tensor_reduce per row
tile).

Sharding: 16 (b, head) pairs across 8 cores -> core c handles b = c // 4,
heads 2*(c%4) and 2*(c%4)+1.  Each core receives h[b] (4 MB) and its two
A matrices; outputs its two (T, d) slices.

Hardware notes (inherited from previous sessions on this toolchain):
  - tensor_tensor_reduce with a PSUM input crashes the device; PSUM is written
    only by PE/ACT here and DVE only copy-reads it.
  - mask/identity constants are DMA'd from host inputs.
"""

import sys

try:
    import concourse.bass  # noqa: F401
except ImportError:  # pragma: no cover
    sys.path.insert(0, "/opt/trn_rl_repo")

import numpy as np

import concourse.bass as bass  # noqa: F401
import concourse.tile as tile
from concourse import bacc, bass_utils, mybir

B, T, D, H = 2, 2048, 512, 8
NCORES = 8
P = 128
NT = T // P      # 16 row tiles
ND = D // P      # 4 contraction chunks
SCH = 512        # s-chunk width (one PSUM bank of fp32)
NS = T // SCH    # 4 column chunks
NEG = -1.0e30

f32 = mybir.dt.float32
f32r = mybir.dt.float32r
bf16 = mybir.dt.bfloat16
EXP = mybir.ActivationFunctionType.Exp
MULT = mybir.AluOpType.mult
ADD = mybir.AluOpType.add
AXX = mybir.AxisListType.X


def build_nc():
    nc = bacc.Bacc("TRN2", target_bir_lowering=False, debug=False)
    hb = nc.dram_tensor("hb", [T, D], f32, kind="ExternalInput")
    A2 = nc.dram_tensor("A2", [2, D, D], f32, kind="ExternalInput")
    cmaskd = nc.dram_tensor("cmaskd", [P, P], f32, kind="ExternalInput")
    identd = nc.dram_tensor("identd", [P, P], f32, kind="ExternalInput")
    out2 = nc.dram_tensor("out2", [2, T, D], f32, kind="ExternalOutput")
    hb_t = hb[:].rearrange("(n p) d -> p n d", p=P)      # [128, 16, 512]
    A2r = A2[:].rearrange("h (c p) e -> p h c e", p=P)   # [128, 2, 4, 512]

    with tile.TileContext(nc) as tc:
        with (
            tc.tile_pool(name="const", bufs=1) as constp,
            tc.tile_pool(name="big", bufs=1) as big,
            tc.tile_pool(name="psum", bufs=8, space="PSUM") as psum,
            tc.tile_pool(name="scr", bufs=3) as scr,
            tc.tile_pool(name="stats", bufs=6) as stats,
            tc.tile_pool(name="outp", bufs=3) as outp,
        ):
            ident = constp.tile([P, P], f32)
            nc.gpsimd.dma_start(out=ident, in_=identd[:])
            cmaskf = constp.tile([P, P], f32)
            nc.gpsimd.dma_start(out=cmaskf, in_=cmaskd[:])
            identb = constp.tile([P, P], bf16)
            nc.vector.tensor_copy(identb, ident)
            cmaskb = constp.tile([P, P], bf16)
            nc.vector.tensor_copy(cmaskb, cmaskf)
            ident_r = ident.bitcast(f32r)

            # The sim serializes all DMA transfers on one device, so emission
            # order ~= arrival order: h rows 0-3 first (transposes start),
            # then A (stage1), then the rest of h.
            A_r = big.tile([P, 2, ND, D], f32r)
            h_all = big.tile([P, NT, D], f32)
            dmaq = [nc.sync, nc.scalar]
            for i in range(4):
                dmaq[i % 2].dma_start(out=h_all[:, i, :], in_=hb_t[:, i, :])
            # A as f32r bits (PE rounds on read): A_r[p, hd, c, e] = A[hd, c*128+p, e]
            for hd in range(2):
                for half in range(2):
                    dmaq[half].dma_start(
                        out=A_r[:, hd, 2 * half : 2 * half + 2].bitcast(f32),
                        in_=A2r[:, hd, 2 * half : 2 * half + 2],
                    )
            for i in range(4, NT):
                dmaq[i % 2].dma_start(out=h_all[:, i, :], in_=hb_t[:, i, :])

            # h^T: hTr[p, c, t] = h[t, c*128 + p]   (f32r bits)
            hTr = big.tile([P, ND, T], f32r)
            # gT = A^T @ hT per head: gTh[p, ec, t] = g[t, ec*128 + p]
            gTh = [big.tile([P, ND, T], f32r, name=f"gTh{hd}") for hd in range(2)]

            def transpose_rowtile(i):
                pt = psum.tile([P, ND, P], f32r, tag="ps")
                for c in range(ND):
                    nc.tensor.transpose(
                        pt[:, c, :],
                        h_all[:, i, c * P : (c + 1) * P].bitcast(f32r),
                        ident_r,
                    )
                # one 512-wide copy distributes the 4 chunks into hTr
                nc.vector.tensor_copy(
                    hTr[:, :, i * P : (i + 1) * P].bitcast(f32), pt.bitcast(f32)
                )

            def stage1(hd, tsl):
                ts_ = slice(tsl * SCH, (tsl + 1) * SCH)
                for ec in range(ND):
                    pg = psum.tile([P, SCH], f32, tag="ps")
                    for k in range(ND):
                        nc.tensor.matmul(
                            pg,
                            A_r[:, hd, k, ec * P : (ec + 1) * P],
                            hTr[:, k, ts_],
                            start=(k == 0),
                            stop=(k == ND - 1),
                        )
                    nc.vector.tensor_copy(gTh[hd][:, ec, ts_].bitcast(f32), pg)

            def stage2_rowtile(hd, i):
                nch = i // 4 + 1
                its = slice(i * P, (i + 1) * P)
                dcol = (i % 4) * P
                wlast = dcol + P          # causal width within last chunk
                w_mm = max(wlast, 2 * P)  # f32r needs moving dim >= 256
                lp = stats.tile([P, 4], f32, tag="lp")
                negd = stats.tile([P, 1], f32, tag="negd")
                chunks = []
                # diag chunk FIRST so negd is ready while PE does the rest
                for idx, j in enumerate([nch - 1] + list(range(nch - 1))):
                    last = j == nch - 1
                    w = w_mm if last else SCH
                    wc = wlast if last else SCH
                    ps = psum.tile([P, SCH], f32, tag="ps")
                    for k in range(ND):
                        nc.tensor.matmul(
                            ps[:, :w],
                            gTh[hd][:, k, its],
                            hTr[:, k, j * SCH : j * SCH + w],
                            start=(k == 0),
                            stop=(k == ND - 1 and not last),
                        )
                    if last:
                        # causal mask added inside PSUM: += I @ cmask
                        nc.tensor.matmul(
                            ps[:, dcol : dcol + P],
                            identb,
                            cmaskb,
                            start=False,
                            stop=True,
                        )
                        # extract -s_tt (diag of the masked block)
                        scd = scr.tile([P, P], f32, tag="scd")
                        nc.vector.tensor_copy(scd, ps[:, dcol : dcol + P])
                        tto = scr.tile([P, P], f32, tag="tto")
                        nc.vector.tensor_tensor_reduce(
                            out=tto,
                            in0=scd,
                            in1=ident,
                            scale=-1.0,
                            scalar=0.0,
                            op0=MULT,
                            op1=ADD,
                            accum_out=negd,
                        )
                    chunks.append((ps, wc, idx))
                for ps, wc, idx in chunks:
                    nc.scalar.activation(
                        out=ps[:, :wc],
                        in_=ps[:, :wc],
                        func=EXP,
                        bias=negd,
                        scale=1.0,
                        accum_out=lp[:, idx : idx + 1],
                    )
                lsum = stats.tile([P, 1], f32, tag="lsum")
                nc.vector.reduce_sum(out=lsum, in_=lp[:, :nch], axis=AXX)
                datt = stats.tile([P, 1], f32, tag="datt")
                nc.vector.reciprocal(datt, lsum)
                ot = outp.tile([P, D], f32, tag="ot")
                nc.vector.tensor_scalar_mul(ot, h_all[:, i, :], datt)
                nc.sync.dma_start(out=out2[hd, its, :], in_=ot)

            # Schedule: group g+1's transposes/stage1 are spread BETWEEN group
            # g's stage2 rows.  The PE cost model punishes just-in-time
            # dependencies (every wait resets the p-state ramp), so each PE
            # instruction's inputs must be produced well ahead: the DVE's
            # hTr/gTh copies for group g+1 get multiple stage2-rows of
            # lead time before PE reaches the group g+1 matmuls.
            for i in range(4):
                transpose_rowtile(i)
            for hd in range(2):
                stage1(hd, 0)
            for tsl in range(NS):
                nxt = []
                if tsl + 1 < NS:
                    nxt = [
                        lambda k=k: transpose_rowtile(4 * (tsl + 1) + k)
                        for k in range(4)
                    ] + [lambda hd=hd, t=tsl + 1: stage1(hd, t) for hd in range(2)]
                rows = [(hd, i) for hd in range(2) for i in range(4 * tsl, 4 * tsl + 4)]
                for idx, (hd, i) in enumerate(rows):
                    stage2_rowtile(hd, i)
                    if idx < len(nxt):
                        nxt[idx]()

    nc.compile()
    return nc


_NC_CACHE = {}


def _get_nc():
    if "nc" not in _NC_CACHE:
        _NC_CACHE["nc"] = build_nc()
    return _NC_CACHE["nc"]


def _consts():
    cmask = np.triu(np.full((P, P), NEG, np.float32), 1)
    ident = np.eye(P, dtype=np.float32)
    return cmask, ident


def make_in_maps(h, A):
    h = np.ascontiguousarray(h, dtype=np.float32)
    A = np.ascontiguousarray(A, dtype=np.float32)
    cmask, ident = _consts()
    in_maps = []
    for c in range(NCORES):
        b = c // 4
        h0 = 2 * (c % 4)
        in_maps.append({"hb": h[b], "A2": np.ascontiguousarray(A[h0 : h0 + 2]),
                        "cmaskd": cmask, "identd": ident})
    return in_maps


def assemble(results):
    full = np.empty((B, H, T, D), dtype=np.float32)
    for c in range(NCORES):
        b = c // 4
        h0 = 2 * (c % 4)
        o = results[c]["out2"]
        full[b, h0] = o[0]
        full[b, h0 + 1] = o[1]
    return full.reshape(B, T, H * D)


def kernel(h, A):
    nc = _get_nc()
    res = bass_utils.run_bass_kernel_spmd(
        nc, make_in_maps(h, A), core_ids=list(range(NCORES))
    )
    return assemble(res.results)
